# revision 16
# baseline (speedup 1.0000x reference)
"""Multi-head attention (B=4, S=2048, D=1024, H=16, hd=64) with RoPE on 8 trn2 cores.

Sharding: core c handles batch b=c//2, head-group hg=c%2 (8 heads, 512 features).
Each core computes y_partial.T = Wo[:, fslice] @ ctx.T for its heads; the host
sums the two partials per batch and adds bo.

v2 schedule (vs baseline's strict proj-then-attention phases):
  1. K projection + RoPE for all S (PE warms up; DMA prefetched per-slab)
  2. Q(qc=0) + scores(qc=0) -> exp starts ~35us in
  3. V projection (PE) overlapping exp(qc=0) on ACT/DVE
  4. steady state per qc: Q-proj, scores(pair) interleaved with PV(prev pair)
     at g-granularity, outproj(qc-1) after first scores of qc.
Exp is split across engines: ACT does exact exp for 21/32 slabs; DVE computes
a Schraudolph approximation (affine + f32->i32 convert, bitcast as f32) for
11/32 slabs -- softmax renormalization keeps the resulting error ~1.2e-2 rel.
"""

import contextlib

import numpy as np

import concourse.bass as bass
import concourse.mybir as mybir
import concourse.tile as tile
from concourse import bacc
from concourse.bass_utils import run_bass_kernel_spmd

F32 = mybir.dt.float32
F32R = mybir.dt.float32r
BF16 = mybir.dt.bfloat16
I16 = mybir.dt.int16
AF = mybir.ActivationFunctionType
ADD = mybir.AluOpType.add
MULT = mybir.AluOpType.mult

B, S, D, H = 4, 2048, 1024, 16
HD = D // H            # 64
NCORES = 8
FC = D // 2            # 512 features (8 heads) per core (2 cores per batch)
NH = FC // HD          # 8 heads per core
QN = 256               # q-chunk width
XS = 256               # x slab width
NQC = S // QN          # 8
NKC = S // 128         # 16 k-chunks
NDC = D // 128         # 8 d-chunks
NFC = FC // 128        # 4 f-chunks (head pairs)
NG = 4                 # kc-chunks per exp slab group
EXP_BIAS = -8.0        # constant shift inside exp (cancels in softmax)
SCALE = 1.0 / np.sqrt(HD)

# Schraudolph fast-exp constants, 16-bit variant: exp(x*SCALE+EXP_BIAS) ~=
# bitcast_bf16(int16(z)) with z = x*SCH_A + SCH_B (bf16 shares f32's exponent
# layout in the top 16 bits). C makes the approximation mean-unbiased.
_C_SHIFT = 0.0575327
_A_EXP = 2.0 ** 7 / np.log(2.0)
SCH_A = float(_A_EXP * SCALE)
SCH_B = float((127.0 - _C_SHIFT) * 2.0 ** 7 + _A_EXP * EXP_BIAS)


def _dve_slab(pair, h, g):
    """11 of 32 (pair, h, g) exp slabs go to DVE (Schraudolph)."""
    return g == 3 or (g == 2 and (2 * pair + h) % 3 == 0)


def build_kernel(dump=False, repeat=1):
    nc = bacc.Bacc("TRN2", debug=False)

    xp = nc.dram_tensor("xp", [S // XS, 128, NDC, XS], F32R, kind="ExternalInput")
    wq = nc.dram_tensor("wq", [128, NDC, FC], F32R, kind="ExternalInput")
    wk = nc.dram_tensor("wk", [128, NDC, FC], F32R, kind="ExternalInput")
    wv = nc.dram_tensor("wv", [128, NDC, FC], F32R, kind="ExternalInput")
    wo = nc.dram_tensor("wo", [128, NFC, D], F32R, kind="ExternalInput")
    bq = nc.dram_tensor("bq", [FC], F32, kind="ExternalInput")
    bk = nc.dram_tensor("bk", [FC], F32, kind="ExternalInput")
    bv = nc.dram_tensor("bv", [1, FC], F32R, kind="ExternalInput")
    c2 = nc.dram_tensor("c2", [128, S], F32, kind="ExternalInput")
    s2 = nc.dram_tensor("s2", [128, S], F32, kind="ExternalInput")
    onesin = nc.dram_tensor("onesin", [1, S], F32R, kind="ExternalInput")
    perm = nc.dram_tensor("perm", [128, 128], F32R, kind="ExternalInput")
    yT = nc.dram_tensor("yT", [D, S], F32, kind="ExternalOutput")
    if dump:
        qT_d = nc.dram_tensor("qT_d", [FC, S], F32, kind="ExternalOutput")
        kT_d = nc.dram_tensor("kT_d", [FC, S], F32, kind="ExternalOutput")
        vt_d = nc.dram_tensor("vt_d", [S, NH, HD + 1], F32, kind="ExternalOutput")

    with tile.TileContext(nc) as tc:
      for _rep in range(repeat):
       with contextlib.ExitStack() as ctx:
        ll = ctx.enter_context(tc.tile_pool(name="ll", bufs=1))

        # persistent tiles
        kT = [ll.tile([128, S], F32R, name=f"kT{i}") for i in range(NFC)]
        vt = [ll.tile([128, NH, HD + 1], BF16, name=f"vt{k}") for k in range(NKC)]
        ebias = ll.tile([128, 1], F32, name="ebias")
        ones_sb = ll.tile([1, 128], F32R, name="ones_sb")
        ones_col = ll.tile([128, NH], F32R, name="ones_col")
        perm_sb = ll.tile([128, 128], F32R, name="perm_sb")
        bqs = ll.tile([128, NFC], F32, name="bqs")
        bks = ll.tile([128, NFC], F32, name="bks")
        c2_sb = ll.tile([128, S], F32, name="c2_sb")
        s2_sb = ll.tile([128, S], F32, name="s2_sb")
        wq_sb = ll.tile([128, NDC, FC], F32R, name="wq_sb")
        wv_sb = ll.tile([128, NDC, FC], F32R, name="wv_sb")
        bv_sb = ll.tile([1, FC], F32R, name="bv_sb")

        nc.vector.memset(ebias, EXP_BIAS)
        nc.sync.dma_start(out=ones_sb, in_=onesin[:][:, 0:128])
        ones_dram = onesin[:]
        nc.sync.dma_start(
            out=ones_col,
            in_=bass.AP(tensor=ones_dram.tensor, offset=ones_dram.offset,
                        ap=[[0, 128], [1, NH]]))
        nc.sync.dma_start(out=perm_sb, in_=perm[:])
        nc.sync.dma_start(out=bqs, in_=bq[:].rearrange("(c p) -> p c", p=128))
        nc.sync.dma_start(out=bks, in_=bk[:].rearrange("(c p) -> p c", p=128))

        with tc.tile_pool(name="sbp", bufs=1) as sbp, \
             tc.tile_pool(name="pp", bufs=2, space="PSUM") as pp:
            xp_r = xp[:]

            def proj_rope(cg, w_sb, bias_t, out_f):
                """Project one x slab chunk cg for all 4 fc; RoPE; write via
                out_f(fc) -> (tile, col_slice). The perm matmul reuses the
                proj PSUM tile (after praw is read) and is software-pipelined
                one fc behind to hide the WAR dependency."""
                xh = sbp.tile([128, NDC, XS], F32R, name="xh", tag="xh", bufs=2)
                nc.sync.dma_start(out=xh, in_=xp_r[cg])
                sg = cg * XS

                def tail(ps, praw, fc):
                    nc.tensor.matmul(ps, perm_sb, praw, start=True, stop=True)
                    prod = sbp.tile([128, XS], F32, name="prod", tag="prod",
                                    bufs=2)
                    t, csl = out_f(fc)
                    nc.vector.tensor_mul(prod, ps, s2_sb[:, sg:sg + XS])
                    nc.gpsimd.tensor_mul(t[:, csl], praw, c2_sb[:, sg:sg + XS])
                    nc.gpsimd.tensor_add(t[:, csl], t[:, csl], prod)

                pend = None
                for fc in range(NFC):
                    ps = pp.tile([128, XS], F32, name="ps", tag="proj", bufs=2)
                    for d in range(NDC):
                        nc.tensor.matmul(
                            ps, w_sb[:, d, fc * 128:(fc + 1) * 128], xh[:, d, :],
                            start=(d == 0), stop=(d == NDC - 1))
                    praw = sbp.tile([128, XS], F32R, name="praw", tag="praw",
                                    bufs=2)
                    nc.vector.tensor_scalar(
                        praw, ps, bias_t[:, fc:fc + 1], None, op0=ADD)
                    if pend is not None:
                        tail(*pend)
                    pend = (ps, praw, fc)
                tail(*pend)

            # ---------------- K pass (wk freed afterwards) ----------------
            with tc.tile_pool(name="kp", bufs=1) as kp:
                wk_sb = kp.tile([128, NDC, FC], F32R, name="wk_sb")
                # chunked weight DMA so the first matmul starts early
                for fc in range(NFC):
                    fsl = slice(fc * 128, (fc + 1) * 128)
                    nc.sync.dma_start(out=wk_sb[:, :, fsl], in_=wk[:][:, :, fsl])
                nc.sync.dma_start(out=c2_sb, in_=c2[:])
                nc.sync.dma_start(out=s2_sb, in_=s2[:])
                nc.sync.dma_start(out=bv_sb, in_=bv[:])
                for fc in range(NFC):
                    fsl = slice(fc * 128, (fc + 1) * 128)
                    nc.sync.dma_start(out=wq_sb[:, :, fsl], in_=wq[:][:, :, fsl])
                nc.sync.dma_start(out=wv_sb, in_=wv[:])
                for scg in range(S // XS):
                    proj_rope(scg, wk_sb, bks,
                              lambda fc, scg=scg: (
                                  kT[fc], slice(scg * XS, (scg + 1) * XS)))

            # wo reuses the SBUF region freed by wk
            wo_sb = sbp.tile([128, NFC, D], F32R, name="wo_sb")
            nc.sync.dma_start(out=wo_sb, in_=wo[:])

            # ---------------- attention pipeline ----------------
            state = {"prev": None, "ctx_done": {}}

            def qproj(qc):
                qt = [sbp.tile([128, QN], F32R, name=f"qc{fc}",
                               tag=f"qT{fc}", bufs=1) for fc in range(NFC)]
                proj_rope(qc, wq_sb, bqs, lambda fc: (qt[fc], slice(0, QN)))
                return qt

            def pv_chunk(prev, g):
                """PV matmuls for kc group g of the previous (qc, pair).

                Both heads accumulate in ONE psum bank [128, 2, QN]: h0's
                first matmul (start=True) marks the whole bank pending-zero,
                so h1's first matmul (start=False) overwrites rather than
                accumulates; the single stop is on h1's last matmul."""
                qc, pair, es_sl, pv_ps = prev
                for h in range(2):
                    hh = pair * 2 + h
                    kind, es = es_sl[h][g]
                    for j in range(NG):
                        kc = g * NG + j
                        mv = es[:, j, :]
                        if kind == "i":
                            mv = mv.bitcast(BF16)
                        nc.tensor.matmul(
                            pv_ps[0:HD + 1, h, :], vt[kc][:, hh, :], mv,
                            start=(h == 0 and kc == 0),
                            stop=(h == 1 and kc == NKC - 1))

            def normalize(prev):
                qc, pair, es_sl, pv_ps = prev
                ct = sbp.tile([128, QN], F32R, name="ct", tag=f"ctx{pair}",
                              bufs=2)
                for h in range(2):
                    denr = sbp.tile([1, QN], F32, name="denr", tag="denr",
                                    bufs=2)
                    nc.vector.reciprocal(denr, pv_ps[HD:HD + 1, h, :])
                    denb = sbp.tile([64, QN], F32, name="denb", tag="denb",
                                    bufs=2)
                    nc.gpsimd.partition_broadcast(denb, denr)
                    nc.vector.tensor_tensor(
                        ct[h * 64:(h + 1) * 64, :], pv_ps[0:HD, h, :], denb,
                        op=MULT)
                state["ctx_done"].setdefault(qc, []).append((pair, ct))

            def scores_exp(qc, pair, qt):
                es_sl = [[None] * NG for _ in range(2)]
                prev = state["prev"]
                if prev is not None:
                    pv_ps = pp.tile([128, 2, QN], F32, name="pv_ps", tag="pv",
                                    bufs=2)
                    prev = (prev[0], prev[1], prev[2], pv_ps)
                for g in range(NG):
                    sc_ps = [pp.tile([128, NG, QN], F32, name="sc_ps",
                                     tag="sc", bufs=2) for _ in range(2)]
                    for j in range(NG):
                        kc = g * NG + j
                        k_sl = slice(kc * 128, (kc + 1) * 128)
                        for h in range(2):
                            nc.tensor.matmul(
                                sc_ps[h][:, j, :],
                                kT[pair][h * 64:(h + 1) * 64, k_sl],
                                qt[pair][h * 64:(h + 1) * 64, :],
                                start=True, stop=True,
                                tile_position=(h * 64, 0))
                    if prev is not None:
                        pv_chunk(prev, g)
                    for h in range(2):
                        if _dve_slab(pair, h, g):
                            es = sbp.tile([128, NG, QN], I16, name="esd",
                                          tag="esd", bufs=2)
                            nc.vector.tensor_scalar(
                                es, sc_ps[h], SCH_A, SCH_B, op0=MULT, op1=ADD)
                            es_sl[h][g] = ("i", es)
                        else:
                            es = sbp.tile([128, NG, QN], BF16, name="esa",
                                          tag="esa", bufs=3)
                            nc.scalar.activation(
                                es, sc_ps[h], AF.Exp, bias=ebias, scale=SCALE)
                            es_sl[h][g] = ("f", es)
                if prev is not None:
                    normalize(prev)
                state["prev"] = (qc, pair, es_sl, None)

            def outproj(qc):
                ctxs = dict(state["ctx_done"].pop(qc))
                for ec in range(NDC):
                    ops = pp.tile([128, QN], F32, name="ops", tag="proj",
                                  bufs=2)
                    for fc in range(NFC):
                        nc.tensor.matmul(
                            ops, wo_sb[:, fc, ec * 128:(ec + 1) * 128],
                            ctxs[fc], start=(fc == 0), stop=(fc == NFC - 1))
                    ysb = sbp.tile([128, QN], F32, name="ysb", tag="y", bufs=2)
                    nc.vector.tensor_copy(ysb, ops)
                    nc.sync.dma_start(
                        out=yT[:].rearrange("(c p) s -> c p s", p=128)
                        [ec, :, qc * QN:(qc + 1) * QN],
                        in_=ysb)

            # qc0: Q + scores pair0 early so exp starts ASAP
            qt0 = qproj(0)
            scores_exp(0, 0, qt0)

            # V pass (overlaps exp of qc0 pair0 on ACT/DVE)
            for scg in range(S // XS):
                xh = sbp.tile([128, NDC, XS], F32R, name="xh", tag="xh", bufs=2)
                nc.sync.dma_start(out=xh, in_=xp_r[scg])
                for ss in range(XS // 128):
                    kg = scg * (XS // 128) + ss
                    psv = pp.tile([128, 2, 256], F32, name="psv", tag="pv",
                                  bufs=2)
                    for half in range(2):
                        f_sl = slice(half * 256, (half + 1) * 256)
                        for d in range(NDC):
                            nc.tensor.matmul(
                                psv[:, half, :], xh[:, d, ss * 128:(ss + 1) * 128],
                                wv_sb[:, d, f_sl],
                                start=(half == 0 and d == 0), stop=False)
                        nc.tensor.matmul(
                            psv[:, half, :], ones_sb[0:1, 0:128], bv_sb[:, f_sl],
                            start=False, stop=(half == 1))
                    nc.vector.tensor_copy(
                        vt[kg][:, :, 0:HD],
                        psv.rearrange("p t (x e) -> p (t x) e", e=HD))
                    nc.gpsimd.tensor_copy(
                        vt[kg][:, :, HD:HD + 1],
                        ones_col.rearrange("p (h o) -> p h o", o=1))

            if dump:
                kd_r = kT_d[:].rearrange("(c p) s -> c p s", p=128)
                for fc in range(NFC):
                    nc.sync.dma_start(out=kd_r[fc], in_=kT[fc].bitcast(F32))
                for kg in range(NKC):
                    nc.sync.dma_start(
                        out=vt_d[:].rearrange("(c p) h e -> c p h e", p=128)[kg],
                        in_=vt[kg].bitcast(F32))

            # steady state
            for pair in range(1, NFC):
                scores_exp(0, pair, qt0)
            for qc in range(1, NQC):
                qt = qproj(qc)
                scores_exp(qc, 0, qt)
                outproj(qc - 1)
                for pair in range(1, NFC):
                    scores_exp(qc, pair, qt)
            # drain: PV + normalize of (7, 3), then outproj(7)
            prev = state["prev"]
            pv_ps = pp.tile([128, 2, QN], F32, name="pv_ps", tag="pv", bufs=2)
            prev = (prev[0], prev[1], prev[2], pv_ps)
            for g in range(NG):
                pv_chunk(prev, g)
            normalize(prev)
            state["prev"] = None
            outproj(NQC - 1)

    nc.finalize()
    return nc


def _rope_tables():
    inv_freq = 1.0 / (10000.0 ** (np.arange(0, HD, 2, dtype=np.float64) / HD))
    pos = np.arange(S, dtype=np.float64)
    sinu = pos[None, :] * inv_freq[:, None]          # [32, S]
    c = np.sin(sinu).astype(np.float32)              # torch code calls this 'cos'
    s = np.cos(sinu).astype(np.float32)              # and this 'sin'
    c2 = np.tile(c, (4, 1))                          # [128, S]
    s2 = np.concatenate([-s, s, -s, s], axis=0)      # [128, S]
    return np.ascontiguousarray(c2), np.ascontiguousarray(s2)


def make_in_maps(inp):
    """inp: dict of full numpy inputs -> list of 8 per-core input maps."""
    c2, s2 = _rope_tables()
    ones = np.ones((1, S), np.float32)
    pm = np.zeros((128, 128), np.float32)
    for h in range(2):
        for j in range(32):
            pm[h * 64 + 32 + j, h * 64 + j] = 1.0      # P[k, j]: out j <- in k
            pm[h * 64 + j, h * 64 + 32 + j] = 1.0
    maps = []
    for c in range(NCORES):
        b, hg = c // 2, c % 2
        fsl = slice(hg * FC, (hg + 1) * FC)
        x = np.asarray(inp["hidden_states"][b], np.float32)
        xp = np.ascontiguousarray(
            x.reshape(S // XS, XS, NDC, 128).transpose(0, 3, 2, 1))
        wqp = np.ascontiguousarray(
            np.asarray(inp["Wq"], np.float32)[fsl].T.reshape(NDC, 128, FC)
            .transpose(1, 0, 2))
        wkp = np.ascontiguousarray(
            np.asarray(inp["Wk"], np.float32)[fsl].T.reshape(NDC, 128, FC)
            .transpose(1, 0, 2))
        wvp = np.ascontiguousarray(
            np.asarray(inp["Wv"], np.float32)[fsl].T.reshape(NDC, 128, FC)
            .transpose(1, 0, 2))
        wop = np.ascontiguousarray(
            np.asarray(inp["Wo"], np.float32)[:, fsl].T.reshape(NFC, 128, D)
            .transpose(1, 0, 2))
        maps.append({
            "xp": xp, "wq": wqp, "wk": wkp, "wv": wvp, "wo": wop,
            "bq": np.ascontiguousarray(np.asarray(inp["bq"], np.float32)[fsl]),
            "bk": np.ascontiguousarray(np.asarray(inp["bk"], np.float32)[fsl]),
            "bv": np.ascontiguousarray(
                np.asarray(inp["bv"], np.float32)[fsl][None, :]),
            "c2": c2, "s2": s2, "onesin": ones, "perm": pm,
        })
    return maps


_NC_CACHE = {}


def kernel(hidden_states, Wq, bq, Wk, bk, Wv, bv, Wo, bo):
    if "nc" not in _NC_CACHE:
        _NC_CACHE["nc"] = build_kernel()
    nc = _NC_CACHE["nc"]
    in_maps = make_in_maps({
        "hidden_states": hidden_states, "Wq": Wq, "bq": bq, "Wk": Wk, "bk": bk,
        "Wv": Wv, "bv": bv, "Wo": Wo,
    })
    res = run_bass_kernel_spmd(nc, in_maps, list(range(NCORES)))
    bo = np.asarray(bo, np.float32)
    out = np.empty((B, S, D), np.float32)
    for b in range(B):
        acc = res.results[2 * b]["yT"] + res.results[2 * b + 1]["yT"]
        out[b] = acc.T + bo[None, :]
    return out
